# revision 53
# baseline (speedup 1.0000x reference)
"""Trainium2 Bass kernel for the AFT-style attention module (v4).

Reference math (per batch element, S=4096, D=1024, H=16, dh=64):
    q = x@Wq+bq ; k = x@Wk+bk ; v = x@Wv+bv
    aw    = softmax(((q@Wa+ba)*s).T + mask)          # [H,S]
    q_av  = blockdiag(aw @ q)                        # [D]
    p     = k * q_av
    bw    = softmax(((p@Wb+bb)*s).T + mask)          # [H,S]
    p_av  = blockdiag(bw @ p)                        # [D]
    attn  = ((p_av * v)@Wu+bu + q) @ Wo + bo
    out   = LayerNorm(x + attn) * ln_g + ln_b

Algebraic refactor (exact; 2.3e-7 vs reference in f64):
    ascore = x @ Wqa + ca        Wqa=(Wq@Wa)*s, ca=(bq@Wa)*s+ba      (host)
    q_av   = blockdiag((aw@x) @ Wq + bq)
    bscore = x @ Wkb + cb        Wkb=(Wk . q_av) @ (Wb*s)          (device)
    p_av   = q_av * blockdiag((bw@x) @ Wk + bk)
    y      = x @ W_big + crow                  (residual folded: +I)
      W_big = (WvT.T . p_av) @ (Wu@Wo) + (Wq@Wo + I)               (host)
      crow  = (bv*p_av)@(Wu@Wo) + bu@Wo + bo
    out    = LN(y)*ln_g + ln_b

Five [S,D]@[D,D] streaming GEMMs collapse to ONE (+ one runtime [D,D]@[D,D]
and tiny score/pool matmuls).  bf16 operands / fp32 PSUM; pooling rhs is
fp8 (q_av/p_av influence the output at the 1e-4 level).  Simulated rel-err
2.9e-3 (gate 2e-2).  All wide matmuls use N=1024 bf16 moving operands
(one instruction per 2-bank psum row) to amortize the ~219-cycle
per-matmul overhead.  LN statistics run on the scalar engine via
activation accumulators; softmax is unnormalized exp with 1/sum folded
into the pooled rows.

Sharding: pure data-parallel, batch B=8 -> 8 NeuronCores, no collectives.
"""

import os

os.environ.setdefault("MYCRO_LOCAL_CACHE", "1")

import sys

if "/opt/trn_rl_repo" not in sys.path:
    sys.path.insert(0, "/opt/trn_rl_repo")

import numpy as np

S = 4096
D = 1024
H = 16
DH = 64
P = 128
NB = D // P          # 8 d-blocks of 128
SP = S // P          # 32 s-blocks of 128
SC = 1024            # score/pool sub-chunk (= DMA chunk)
NSC = S // SC        # 4
SPC = SC // P        # 8 s-blocks per sub-chunk
SCALE = float((D / H) ** -0.5)   # 0.125
EPS = 1e-6
NCORES = 8

LAST_EXEC_TIME_NS = None
_COMPILED = {}


def _build(no_crow):
    import concourse.bass as bass
    import concourse.mybir as mybir
    import concourse.tile as tile
    from concourse import bacc
    from concourse.masks import make_identity
    from contextlib import ExitStack

    FP = mybir.dt.float32
    BF = mybir.dt.bfloat16
    F8 = mybir.dt.float8e4
    AL = mybir.AluOpType
    AF = mybir.ActivationFunctionType

    nc = bacc.Bacc("TRN2", target_bir_lowering=False, debug=False)

    # ---------------- external I/O (per-core shard shapes) ----------------
    xT_d = nc.declare_dram_parameter("xT16", [P, NB, S], BF, isOutput=False)
    xn_d = nc.declare_dram_parameter("xn8", [P, SP, D], F8, isOutput=False)
    mask_d = nc.declare_dram_parameter("mask16", [1, S], BF, isOutput=False)
    wqa_d = nc.declare_dram_parameter("wqa", [P, NB, H], BF, isOutput=False)
    wbs_d = nc.declare_dram_parameter("wbs", [P, NB, H], BF, isOutput=False)
    ca_d = nc.declare_dram_parameter("ca", [H, 1], FP, isOutput=False)
    bb_d = nc.declare_dram_parameter("bb", [H, 1], FP, isOutput=False)
    # fp8 weights on the q_av/p_av/correction paths (host-scaled; the
    # inverse scales fold into drain ops)
    wq_d = nc.declare_dram_parameter("wq8", [P, NB, D], F8, isOutput=False)
    wk_d = nc.declare_dram_parameter("wk8", [P, NB, D], F8, isOutput=False)
    wkT_d = nc.declare_dram_parameter("wkT8", [P, NB, D], F8, isOutput=False)
    wvT_d = nc.declare_dram_parameter("wvT8", [P, NB, D], F8, isOutput=False)
    wuwo_d = nc.declare_dram_parameter("wuwo8", [P, NB, D], F8, isOutput=False)
    wqwo_d = nc.declare_dram_parameter("wqwo16", [P, NB, D], BF, isOutput=False)
    bqP_d = nc.declare_dram_parameter("bqP", [P, NB], FP, isOutput=False)
    bkP_d = nc.declare_dram_parameter("bkP", [P, NB], FP, isOutput=False)
    bk16_d = nc.declare_dram_parameter("bk16P", [P, NB], BF, isOutput=False)
    bv16_d = nc.declare_dram_parameter("bv16P", [P, NB], BF, isOutput=False)
    buwobo_d = nc.declare_dram_parameter("buwobo", [1, D], FP, isOutput=False)
    lng_d = nc.declare_dram_parameter("ln_g", [1, D], FP, isOutput=False)
    lnb_d = nc.declare_dram_parameter("ln_b", [1, D], FP, isOutput=False)
    out_d = nc.declare_dram_parameter("out", [S, D], FP, isOutput=True)

    with tile.TileContext(nc) as tc, ExitStack() as ctx:
        # ------------- whole-kernel pools -------------
        consts = ctx.enter_context(tc.tile_pool(name="consts", bufs=1))
        small = ctx.enter_context(tc.tile_pool(name="small", bufs=2))

        xT = consts.tile([P, NB, S], BF, tag="xT")
        xn = consts.tile([P, SP, D], F8, tag="xn")
        wbig = consts.tile([P, NB, D], BF, tag="wbig")
        wup16 = consts.tile([P, NB, D], BF, tag="wup16")
        crowf = consts.tile([1, D], BF, tag="crowf")
        lng_b = consts.tile([P, D], BF, tag="lng")
        lnb_b = consts.tile([P, D], BF, tag="lnb")
        qav = consts.tile([P, NB], FP, tag="qav")
        kav = consts.tile([P, NB], FP, tag="kav")
        pav = consts.tile([P, NB], FP, tag="pav")
        bv16 = consts.tile([P, NB], BF, tag="bv16")
        id_bf = consts.tile([P, P], BF, tag="id_bf")
        make_identity(nc, id_bf[:])
        ones16 = consts.tile([1, H], BF, tag="ones16")
        nc.vector.memset(ones16[:], 1.0)
        ones128 = consts.tile([1, P], BF, tag="ones128")
        nc.vector.memset(ones128[:], 1.0)
        eps_t = consts.tile([P, 1], FP, tag="eps")
        nc.vector.memset(eps_t[:], EPS)

        # =========================================================
        # Phases A-C
        # =========================================================
        with tc.tile_pool(name="wpool", bufs=3) as wp:
          with tc.tile_pool(name="phAB", bufs=1) as phab, \
               tc.tile_pool(name="spa1", bufs=1) as spa1, \
               tc.tile_pool(name="sp2", bufs=2) as sp2:

            # ---- small parameter loads (gpsimd queue) ----
            awT = phab.tile([P, SP, H], BF, tag="awT")
            bwT = phab.tile([P, SP, H], BF, tag="bwT")
            asums = phab.tile([H, 2 * NSC], FP, tag="asums")
            bsums = phab.tile([H, 2 * NSC], FP, tag="bsums")
            mask16 = phab.tile([1, S], BF, tag="mask16")
            nc.sync.dma_start(out=mask16[:], in_=mask_d[:])
            wqa = phab.tile([P, NB, H], BF, tag="wqa")
            nc.gpsimd.dma_start(out=wqa[:], in_=wqa_d[:])
            wbs = phab.tile([P, NB, H], BF, tag="wbs")
            nc.gpsimd.dma_start(out=wbs[:], in_=wbs_d[:])
            ca = phab.tile([H, 1], FP, tag="ca")
            nc.gpsimd.dma_start(out=ca[:], in_=ca_d[:])
            bb = phab.tile([H, 1], FP, tag="bb")
            nc.gpsimd.dma_start(out=bb[:], in_=bb_d[:])
            bqP = phab.tile([P, NB], FP, tag="bqP")
            nc.gpsimd.dma_start(out=bqP[:], in_=bqP_d[:])
            bkP = phab.tile([P, NB], FP, tag="bkP")
            nc.gpsimd.dma_start(out=bkP[:], in_=bkP_d[:])
            bk16 = phab.tile([P, NB], BF, tag="bk16")
            nc.gpsimd.dma_start(out=bk16[:], in_=bk16_d[:])
            nc.gpsimd.dma_start(out=bv16[:], in_=bv16_d[:])
            buwobo = phab.tile([1, D], FP, tag="buwobo")
            nc.gpsimd.dma_start(out=buwobo[:], in_=buwobo_d[:])
            for src, dst in ((lng_d, lng_b), (lnb_d, lnb_b)):
                t = phab.tile([1, D], FP, tag="lrow")
                nc.gpsimd.dma_start(out=t[:], in_=src[:])
                t16 = phab.tile([1, D], BF, tag="lrow16")
                nc.vector.tensor_copy(t16[:], t[:])
                nc.gpsimd.partition_broadcast(dst[:], t16[:1, :])

            # big-weight rotation (3 slots): wq(0) wkT(1) wk(2) wvT(0) wuwo(1)
            wq8 = wp.tile([P, NB, D], F8, tag="w")
            nc.gpsimd.dma_start(out=wq8[:], in_=wq_d[:])
            wkT8 = wp.tile([P, NB, D], F8, tag="w")
            nc.gpsimd.dma_start(out=wkT8[:], in_=wkT_d[:])

            # ---- helpers ----
            def score_sub(wsc, biast, sums, c2, ps_sc, awpool):
                # two independent 512-wide halves in rotating psum banks so
                # the next half's matmuls never WAR-wait on the prior exp
                awcs = []
                for hf in range(2):
                    o = c2 * SC + hf * (SC // 2)
                    ps = ps_sc.tile([H, SC // 2], FP, tag="sc")
                    for k in range(NB):
                        nc.tensor.matmul(
                            ps[:], wsc[:, k, :], xT[:, k, o:o + SC // 2],
                            start=(k == 0), stop=False)
                    nc.tensor.matmul(
                        ps[:], ones16[:1, :], mask16[:1, o:o + SC // 2],
                        start=False, stop=True)
                    awc = awpool.tile([H, SC // 2], BF, tag="awc")
                    nc.scalar.activation(
                        awc[:], ps[:], AF.Exp,
                        bias=biast[:, :1], scale=1.0,
                        accum_out=sums[:, 2 * c2 + hf:2 * c2 + hf + 1])
                    awcs.append(awc)
                return awcs

            def trans_sub(awcs, awT_t, c2, ps_tp):
                for hf in range(2):
                    for i in range(SPC // 2):
                        t = c2 * SPC + hf * (SPC // 2) + i
                        tp = ps_tp.tile([P, H], BF, tag="tp")
                        nc.tensor.matmul(
                            tp[:], awcs[hf][:, i * P:(i + 1) * P],
                            id_bf[:H, :H], is_transpose=True)
                        nc.vector.tensor_copy(awT_t[:, t, :], tp[:])

            def pool_sub(awT_t, pool_ps, c2):
                for i in range(SPC):
                    t = c2 * SPC + i
                    for hf in range(2):
                        o = hf * (D // 2)
                        nc.tensor.matmul(
                            pool_ps[:, o:o + D // 2], awT_t[:, t, :],
                            xn[:, t, o:o + D // 2],
                            start=(t == 0), stop=(t == SP - 1),
                            skip_group_check=True)

            def rinv_of(sums):
                tot = small.tile([H, 1], FP, tag="tot")
                nc.vector.reduce_sum(tot[:], sums[:], axis=mybir.AxisListType.X)
                rinv = small.tile([H, 1], FP, tag="rinv")
                nc.vector.reciprocal(rinv[:], tot[:])
                return rinv

            def pooled_proj_extract(pool_ps, rinv, wnat, badd, av_t,
                                    ps_tp, ps_sc):
                """av = blockdiag((pool/sum) @ Wnat) + badd  -> [P,NB] f32."""
                aXs = spa1.tile([H, D], BF, tag="xrow")
                nc.vector.tensor_scalar_mul(aXs[:], pool_ps[:], rinv[:, :1])
                aXT = spa1.tile([P, NB, H], BF, tag="aXT")
                for j in range(NB):
                    tp = ps_tp.tile([P, H], BF, tag="tp")
                    nc.tensor.matmul(
                        tp[:], aXs[:, j * P:(j + 1) * P], id_bf[:H, :H],
                        is_transpose=True)
                    nc.vector.tensor_copy(aXT[:, j, :], tp[:])
                q2h0 = ps_sc.tile([H, SC // 2], FP, tag="sc")
                q2h1 = ps_sc.tile([H, SC // 2], FP, tag="sc")
                q2h = (q2h0, q2h1)
                for k in range(NB):
                    for hf in range(2):
                        o = hf * (D // 2)
                        nc.tensor.matmul(
                            q2h[hf][:], aXT[:, k, :],
                            wnat[:, k, o:o + D // 2],
                            start=(k == 0), stop=(k == NB - 1))
                q2s = spa1.tile([H, D], BF, tag="xrow")
                for hf in range(2):
                    nc.vector.tensor_scalar_mul(
                        q2s[:, hf * (D // 2):(hf + 1) * (D // 2)], q2h[hf][:],
                        1.0 / 16.0)
                for j in range(NB):
                    tp = ps_tp.tile([P, H], BF, tag="tp")
                    nc.tensor.matmul(
                        tp[:], q2s[:, j * P:(j + 1) * P], id_bf[:H, :H],
                        is_transpose=True)
                    nc.vector.tensor_copy(
                        av_t[0:DH, j:j + 1], tp[0:DH, 2 * j:2 * j + 1])
                    nc.vector.tensor_copy(
                        av_t[DH:P, j:j + 1], tp[DH:P, 2 * j + 1:2 * j + 2])
                nc.vector.tensor_add(av_t[:], av_t[:], badd[:])

            # =====================================================
            # Phases A & B under the score/pool psum pools
            # =====================================================
            with tc.tile_pool(name="ps_sc", bufs=2, space="PSUM") as ps_sc, \
                 tc.tile_pool(name="ps_pl", bufs=1, space="PSUM") as ps_pl, \
                 tc.tile_pool(name="ps_tp", bufs=2, space="PSUM") as ps_tp, \
                 tc.tile_pool(name="ps_wkb", bufs=1, space="PSUM") as ps_wkb, \
                 tc.tile_pool(name="ps_warm", bufs=1, space="PSUM") as ps_wm, \
                 tc.tile_pool(name="sp4", bufs=4) as sp4:

                pool_ps = ps_pl.tile([H, D], FP, tag="pool")
                warm_ps = ps_wm.tile([P, P], FP, tag="warm")

                def keep_warm(n):
                    # dependency-free matmuls that fill in-order queue gaps
                    # so the PE HAM clock gate stays at 8/8 (2.4 GHz)
                    for _ in range(n):
                        nc.tensor.matmul(warm_ps[:], id_bf[:], id_bf[:],
                                         start=True, stop=True,
                                         skip_group_check=True)

                # pre-warm the PE during the initial x DMA window
                keep_warm(48)

                # ---- Phase A: x DMA + ascore + q_av pooling ----
                # software-pipelined: score(c2) issues before trans/pool of
                # c2-1, so the in-order tensor queue never waits on the
                # scalar exp of the chunk it just scored.
                pend = None
                for c2 in range(NSC):
                    lo = c2 * SC
                    hs = SC // 2
                    nc.sync.dma_start(out=xT[:, :, lo:lo + hs],
                                      in_=xT_d.ap()[:, :, lo:lo + hs])
                    nc.sync.dma_start(out=xT[:, :, lo + hs:lo + SC],
                                      in_=xT_d.ap()[:, :, lo + hs:lo + SC])
                    nc.scalar.dma_start(
                        out=xn[:, c2 * SPC:(c2 + 1) * SPC, :],
                        in_=xn_d.ap()[:, c2 * SPC:(c2 + 1) * SPC, :])
                    awc = score_sub(wqa, ca, asums, c2, ps_sc, sp4)
                    if pend is not None:
                        trans_sub(pend[0], awT, pend[1], ps_tp)
                        pool_sub(awT, pool_ps, pend[1])
                    pend = (awc, c2)
                trans_sub(pend[0], awT, pend[1], ps_tp)
                pool_sub(awT, pool_ps, pend[1])

                rinv_a = rinv_of(asums)
                keep_warm(24)
                pooled_proj_extract(pool_ps, rinv_a, wq8, bqP, qav,
                                    ps_tp, ps_sc)
                keep_warm(12)

                # ---- Phase B: bscore (Wkb from q_av) + p_av pooling ----
                wk8 = wp.tile([P, NB, D], F8, tag="w")
                nc.gpsimd.dma_start(out=wk8[:], in_=wk_d[:])

                wbp = spa1.tile([P, NB, H], BF, tag="wbp")
                for j in range(NB):
                    nc.vector.tensor_scalar_mul(
                        wbp[:, j, :], wbs[:, j, :], qav[:, j:j + 1])
                wkbp = ps_wkb.tile([P, NB, H], FP, tag="wkbp")
                for m in range(NB):
                    for k in range(NB):
                        nc.tensor.matmul(
                            wkbp[:, m, :], wkT8[:, k, m * P:(m + 1) * P],
                            wbp[:, k, :],
                            start=(k == 0), stop=(k == NB - 1),
                            skip_group_check=True)
                wkb = spa1.tile([P, NB, H], BF, tag="wkb")
                nc.scalar.activation(wkb[:], wkbp[:], AF.Identity,
                                     scale=1.0 / 16.0)
                cbp = ps_sc.tile([H, SC // 2], FP, tag="sc")
                for k in range(NB):
                    nc.tensor.matmul(
                        cbp[:, :1], wbp[:, k, :], bk16[:, k:k + 1],
                        start=(k == 0), stop=(k == NB - 1))
                cb = small.tile([H, 1], FP, tag="cbt")
                nc.vector.tensor_add(cb[:], cbp[:, :1], bb[:])

                # prefetch phase-C weights into freed rotation slots
                wvT8 = wp.tile([P, NB, D], F8, tag="w")
                nc.gpsimd.dma_start(out=wvT8[:], in_=wvT_d[:])
                wuwo8 = wp.tile([P, NB, D], F8, tag="w")
                nc.gpsimd.dma_start(out=wuwo8[:], in_=wuwo_d[:])

                pend = None
                for c2 in range(NSC):
                    awc = score_sub(wkb, cb, bsums, c2, ps_sc, sp4)
                    if pend is not None:
                        trans_sub(pend[0], bwT, pend[1], ps_tp)
                        pool_sub(bwT, pool_ps, pend[1])
                    pend = (awc, c2)
                trans_sub(pend[0], bwT, pend[1], ps_tp)
                pool_sub(bwT, pool_ps, pend[1])

                rinv_b = rinv_of(bsums)
                keep_warm(24)
                pooled_proj_extract(pool_ps, rinv_b, wk8, bkP, kav,
                                    ps_tp, ps_sc)
                keep_warm(12)
                nc.vector.tensor_mul(pav[:], qav[:], kav[:])

                # WuWo' = (pav/64)-row-scaled fp8 WuWo -> bf16
                pav64 = small.tile([P, NB], FP, tag="pav64")
                nc.vector.tensor_scalar_mul(pav64[:], pav[:], 1.0 / 64.0)
                for j in range(NB):
                    nc.vector.tensor_scalar_mul(
                        wup16[:, j, :], wuwo8[:, j, :], pav64[:, j:j + 1])
                keep_warm(10)
                if not no_crow:
                    crh0 = ps_sc.tile([H, SC // 2], FP, tag="sc")
                    crh1 = ps_sc.tile([H, SC // 2], FP, tag="sc")
                    crh = (crh0, crh1)
                    for k in range(NB):
                        for hf in range(2):
                            o = hf * (D // 2)
                            nc.tensor.matmul(
                                crh[hf][:1, :], bv16[:, k:k + 1],
                                wup16[:, k, o:o + D // 2],
                                start=(k == 0), stop=(k == NB - 1))
                    for hf in range(2):
                        o = hf * (D // 2)
                        nc.vector.tensor_add(crowf[:, o:o + D // 2],
                                             crh[hf][:1, :],
                                             buwobo[:, o:o + D // 2])

          # =====================================================
          # Phase C: W_big = (WvT.T . pav) @ WuWo' + (WqWo + I)
          # =====================================================
          with tc.tile_pool(name="wstream", bufs=2) as ws, \
               tc.tile_pool(name="ps_wb", bufs=2, space="PSUM") as ps_wb:
                for m in range(NB):
                    wqwo_m = ws.tile([P, D], BF, tag="wqwom")
                    nc.gpsimd.dma_start(out=wqwo_m[:],
                                        in_=wqwo_d.ap()[:, m, :])
                    ps = ps_wb.tile([P, D], FP, tag="wbps")
                    for k in range(NB):
                        for hf in range(2):
                            o = hf * (D // 2)
                            nc.tensor.matmul(
                                ps[:, o:o + D // 2],
                                wvT8[:, k, m * P:(m + 1) * P],
                                wup16[:, k, o:o + D // 2],
                                start=(k == 0), stop=(k == NB - 1),
                                skip_group_check=True)
                    nc.vector.scalar_tensor_tensor(
                        wbig[:, m, :], ps[:], 1.0 / 16.0, wqwo_m[:],
                        op0=AL.mult, op1=AL.add)

        # =========================================================
        # Phase D: out = LN(x@(W_big+I) + crow) * g + b
        # =========================================================
        with tc.tile_pool(name="spD", bufs=4) as sp, \
             tc.tile_pool(name="ps_nat", bufs=4, space="PSUM") as ps_nat:
            for sm in range(SP):
                pn = ps_nat.tile([P, D], FP, tag="nat")
                if not no_crow:
                    for hf in range(2):
                        o = hf * (D // 2)
                        nc.tensor.matmul(
                            pn[:, o:o + D // 2], ones128[:1, :],
                            crowf[:1, o:o + D // 2],
                            start=True, stop=False, skip_group_check=True)
                for k in range(NB):
                    for hf in range(2):
                        o = hf * (D // 2)
                        nc.tensor.matmul(
                            pn[:, o:o + D // 2],
                            xT[:, k, sm * P:(sm + 1) * P],
                            wbig[:, k, o:o + D // 2],
                            start=(no_crow and k == 0), stop=(k == NB - 1),
                            skip_group_check=True)
                # LN epilogue split across scalar/vector/gpsimd so no
                # engine exceeds the tensor engine's per-block time
                y16 = sp.tile([P, D], BF, tag="y16")
                sy = small.tile([P, 1], FP, tag="sy")
                nc.scalar.activation(y16[:], pn[:], AF.Identity,
                                     accum_out=sy[:])
                ysq = sp.tile([P, D], BF, tag="ysq")
                s2 = small.tile([P, 1], FP, tag="s2")
                nc.scalar.activation(ysq[:], pn[:], AF.Square,
                                     accum_out=s2[:])
                mean = small.tile([P, 1], FP, tag="mean")
                nc.vector.tensor_scalar_mul(mean[:], sy[:], 1.0 / D)
                msq = small.tile([P, 1], FP, tag="msq")
                nc.vector.tensor_mul(msq[:], mean[:], mean[:])
                var = small.tile([P, 1], FP, tag="var")
                nc.vector.scalar_tensor_tensor(
                    var[:], s2[:], 1.0 / D, msq[:],
                    op0=AL.mult, op1=AL.subtract)
                sq = small.tile([P, 1], FP, tag="sq")
                nc.scalar.activation(sq[:], var[:], AF.Sqrt,
                                     bias=eps_t[:, :1], scale=1.0)
                rstd = small.tile([P, 1], FP, tag="rstd")
                nc.vector.reciprocal(rstd[:], sq[:])
                t16 = sp.tile([P, D], BF, tag="t16")
                nc.vector.scalar_tensor_tensor(
                    t16[:], y16[:], mean[:, :1], lng_b[:],
                    op0=AL.subtract, op1=AL.mult)
                outt = sp.tile([P, D], FP, tag="outt")
                nc.vector.scalar_tensor_tensor(
                    outt[:], t16[:], rstd[:, :1], lnb_b[:],
                    op0=AL.mult, op1=AL.add)
                nc.sync.dma_start(out=out_d[sm * P:(sm + 1) * P, :],
                                  in_=outt[:])

    nc.compile()
    return nc


def _install_ntff_hook_shim():
    """The agent image's antenv lacks axon_hooks, so trace=True degrades.
    Recreate the hook from the boot helper so neuron-profile works."""
    import types
    try:
        import antenv.axon_hooks  # noqa: F401
        return
    except ImportError:
        pass
    try:
        import antenv
        from trn_agent_boot.trn_boot import _ntff_profile_via_ctypes
        hook = _ntff_profile_via_ctypes("/opt/axon/libaxon_pjrt.so")
        mod = types.ModuleType("antenv.axon_hooks")
        mod._hook = hook
        mod.get_axon_ntff_profile_hook = lambda: mod._hook
        mod.set_axon_ntff_profile_hook = lambda h: setattr(mod, "_hook", h)
        sys.modules["antenv.axon_hooks"] = mod
        antenv.axon_hooks = mod
    except Exception as e:  # tracing is best-effort
        print(f"ntff hook shim failed: {e}", file=sys.stderr)


def _get_compiled(no_crow):
    key = ("nc", no_crow)
    if key not in _COMPILED:
        _COMPILED[key] = _build(no_crow)
    return _COMPILED[key]


def kernel(x, mask, Wq, bq, Wk, bk, Wv, bv, Wa, ba, Wb, bb, Wu, bu, Wo, bo,
           ln_g, ln_b):
    global LAST_EXEC_TIME_NS
    import ml_dtypes
    from concourse.bass_utils import run_bass_kernel_spmd

    bf16 = ml_dtypes.bfloat16
    f8 = ml_dtypes.float8_e4m3
    f32 = lambda a: np.ascontiguousarray(np.asarray(a, dtype=np.float32))

    x = f32(x)
    B = x.shape[0]
    assert B == NCORES and x.shape == (B, S, D)
    mask = f32(mask).reshape(B, S)
    Wq, Wk, Wv, Wu, Wo = f32(Wq), f32(Wk), f32(Wv), f32(Wu), f32(Wo)
    Wa, Wb = f32(Wa), f32(Wb)
    bq, bk, bv, ba, bb_, bu, bo = (f32(v).ravel() for v in
                                   (bq, bk, bv, ba, bb, bu, bo))
    ln_g, ln_b = f32(ln_g).reshape(1, D), f32(ln_b).reshape(1, D)

    # ---- host-side weight folding (f32) ----
    Wqa = (Wq @ Wa) * SCALE                      # [D, H]
    ca = ((bq @ Wa) * SCALE + ba).reshape(H, 1)
    WuWo = Wu @ Wo                               # [D, D]
    WqWo = Wq @ Wo + np.eye(D, dtype=np.float32)  # residual folded in
    buwobo = (bu @ Wo + bo).reshape(1, D)

    dmaj = lambda M: np.ascontiguousarray(
        M.reshape(NB, P, -1).transpose(1, 0, 2))     # [D, X] -> [P, NB, X]
    vP = lambda v: np.ascontiguousarray(v.reshape(NB, P).T)  # [D] -> [P, NB]

    shared = {
        "wqa": dmaj(Wqa).astype(bf16),
        "wbs": dmaj(Wb * SCALE).astype(bf16),
        "ca": ca,
        "bb": bb_.reshape(H, 1),
        "wq8": (16.0 * dmaj(Wq)).astype(f8),
        "wk8": (16.0 * dmaj(Wk)).astype(f8),
        "wkT8": (16.0 * dmaj(np.ascontiguousarray(Wk.T))).astype(f8),
        "wvT8": (16.0 * dmaj(np.ascontiguousarray(Wv.T))).astype(f8),
        "wuwo8": (64.0 * dmaj(WuWo)).astype(f8),
        "wqwo16": dmaj(WqWo).astype(bf16),
        "bqP": vP(bq),
        "bkP": vP(bk),
        "bk16P": vP(bk).astype(bf16),
        "bv16P": vP(bv).astype(bf16),
        "buwobo": buwobo,
        "ln_g": ln_g,
        "ln_b": ln_b,
    }

    no_crow = not (np.any(bv) or np.any(bu) or np.any(bo))
    nc = _get_compiled(no_crow)

    in_maps = []
    for i in range(B):
        xT16 = np.ascontiguousarray(x[i].T).astype(bf16)  # [D, S]
        m = {
            "xT16": np.ascontiguousarray(
                xT16.reshape(NB, P, S).transpose(1, 0, 2)),
            "xn8": np.ascontiguousarray(
                x[i].reshape(SP, P, D).transpose(1, 0, 2)).astype(f8),
            "mask16": mask[i:i + 1].astype(bf16),
        }
        m.update(shared)
        in_maps.append(m)

    trace = bool(int(os.environ.get("KERNEL_TRACE", "0")))
    if trace:
        _install_ntff_hook_shim()
    res = run_bass_kernel_spmd(nc, in_maps, core_ids=list(range(NCORES)),
                               trace=trace)
    LAST_EXEC_TIME_NS = res.exec_time_ns
    out = np.stack([res.results[i]["out"] for i in range(B)], axis=0)
    return out.astype(np.float32)


if __name__ == "__main__":
    np.random.seed(0)
    ins = {
        "x": np.random.randn(8, S, D).astype(np.float32),
        "mask": np.zeros((8, 1, S), np.float32),
    }
    std = 0.02
    for n, shp in (("Wq", (D, D)), ("Wk", (D, D)), ("Wv", (D, D)),
                   ("Wa", (D, H)), ("Wb", (D, H)), ("Wu", (D, D)),
                   ("Wo", (D, D))):
        ins[n] = (std * np.random.randn(*shp)).astype(np.float32)
    for n, shp in (("bq", (D,)), ("bk", (D,)), ("bv", (D,)), ("ba", (H,)),
                   ("bb", (H,)), ("bu", (D,)), ("bo", (D,)), ("ln_b", (D,))):
        ins[n] = np.zeros(shp, np.float32)
    ins["ln_g"] = np.ones((D,), np.float32)
    out = kernel(**ins)
    print("out", out.shape, out.dtype, float(np.abs(out).mean()))


# revision 54
# speedup vs baseline: 1.1076x; 1.1076x over previous
"""Trainium2 Bass kernel for the AFT-style attention module (v4).

Reference math (per batch element, S=4096, D=1024, H=16, dh=64):
    q = x@Wq+bq ; k = x@Wk+bk ; v = x@Wv+bv
    aw    = softmax(((q@Wa+ba)*s).T + mask)          # [H,S]
    q_av  = blockdiag(aw @ q)                        # [D]
    p     = k * q_av
    bw    = softmax(((p@Wb+bb)*s).T + mask)          # [H,S]
    p_av  = blockdiag(bw @ p)                        # [D]
    attn  = ((p_av * v)@Wu+bu + q) @ Wo + bo
    out   = LayerNorm(x + attn) * ln_g + ln_b

Algebraic refactor (exact; 2.3e-7 vs reference in f64):
    ascore = x @ Wqa + ca        Wqa=(Wq@Wa)*s, ca=(bq@Wa)*s+ba      (host)
    q_av   = blockdiag((aw@x) @ Wq + bq)
    bscore = x @ Wkb + cb        Wkb=(Wk . q_av) @ (Wb*s)          (device)
    p_av   = q_av * blockdiag((bw@x) @ Wk + bk)
    y      = x @ W_big + crow                  (residual folded: +I)
      W_big = (WvT.T . p_av) @ (Wu@Wo) + (Wq@Wo + I)               (host)
      crow  = (bv*p_av)@(Wu@Wo) + bu@Wo + bo
    out    = LN(y)*ln_g + ln_b

Five [S,D]@[D,D] streaming GEMMs collapse to ONE (+ one runtime [D,D]@[D,D]
and tiny score/pool matmuls).  bf16 operands / fp32 PSUM; pooling rhs is
fp8 (q_av/p_av influence the output at the 1e-4 level).  Simulated rel-err
2.9e-3 (gate 2e-2).  All wide matmuls use N=1024 bf16 moving operands
(one instruction per 2-bank psum row) to amortize the ~219-cycle
per-matmul overhead.  LN statistics run on the scalar engine via
activation accumulators; softmax is unnormalized exp with 1/sum folded
into the pooled rows.

Sharding: pure data-parallel, batch B=8 -> 8 NeuronCores, no collectives.
"""

import os

os.environ.setdefault("MYCRO_LOCAL_CACHE", "1")

import sys

if "/opt/trn_rl_repo" not in sys.path:
    sys.path.insert(0, "/opt/trn_rl_repo")

import numpy as np

S = 4096
D = 1024
H = 16
DH = 64
P = 128
NB = D // P          # 8 d-blocks of 128
SP = S // P          # 32 s-blocks of 128
SC = 1024            # score/pool sub-chunk (= DMA chunk)
NSC = S // SC        # 4
SPC = SC // P        # 8 s-blocks per sub-chunk
SCALE = float((D / H) ** -0.5)   # 0.125
EPS = 1e-6
NCORES = 8

LAST_EXEC_TIME_NS = None
_COMPILED = {}


def _build(no_crow):
    import concourse.bass as bass
    import concourse.mybir as mybir
    import concourse.tile as tile
    from concourse import bacc
    from concourse.masks import make_identity
    from contextlib import ExitStack

    FP = mybir.dt.float32
    BF = mybir.dt.bfloat16
    F8 = mybir.dt.float8e4
    AL = mybir.AluOpType
    AF = mybir.ActivationFunctionType

    nc = bacc.Bacc("TRN2", target_bir_lowering=False, debug=False)

    # ---------------- external I/O (per-core shard shapes) ----------------
    xT_d = nc.declare_dram_parameter("xT16", [P, NB, S], BF, isOutput=False)
    xn_d = nc.declare_dram_parameter("xn8", [P, SP, D], F8, isOutput=False)
    mask_d = nc.declare_dram_parameter("mask16", [1, S], BF, isOutput=False)
    wqa_d = nc.declare_dram_parameter("wqa", [P, NB, H], BF, isOutput=False)
    wbs_d = nc.declare_dram_parameter("wbs", [P, NB, H], BF, isOutput=False)
    ca_d = nc.declare_dram_parameter("ca", [H, 1], FP, isOutput=False)
    bb_d = nc.declare_dram_parameter("bb", [H, 1], FP, isOutput=False)
    # fp8 weights on the q_av/p_av/correction paths (host-scaled; the
    # inverse scales fold into drain ops)
    wq_d = nc.declare_dram_parameter("wq8", [P, NB, D], F8, isOutput=False)
    wk_d = nc.declare_dram_parameter("wk8", [P, NB, D], F8, isOutput=False)
    wkT_d = nc.declare_dram_parameter("wkT8", [P, NB, D], F8, isOutput=False)
    wvT_d = nc.declare_dram_parameter("wvT8", [P, NB, D], F8, isOutput=False)
    wuwo_d = nc.declare_dram_parameter("wuwo8", [P, NB, D], F8, isOutput=False)
    wqwo_d = nc.declare_dram_parameter("wqwo16", [P, NB, D], BF, isOutput=False)
    bqP_d = nc.declare_dram_parameter("bqP", [P, NB], FP, isOutput=False)
    bkP_d = nc.declare_dram_parameter("bkP", [P, NB], FP, isOutput=False)
    bk16_d = nc.declare_dram_parameter("bk16P", [P, NB], BF, isOutput=False)
    bv16_d = nc.declare_dram_parameter("bv16P", [P, NB], BF, isOutput=False)
    buwobo_d = nc.declare_dram_parameter("buwobo", [1, D], FP, isOutput=False)
    lng_d = nc.declare_dram_parameter("ln_g", [1, D], FP, isOutput=False)
    lnb_d = nc.declare_dram_parameter("ln_b", [1, D], FP, isOutput=False)
    out_d = nc.declare_dram_parameter("out", [S, D], FP, isOutput=True)

    with tile.TileContext(nc) as tc, ExitStack() as ctx:
        # ------------- whole-kernel pools -------------
        consts = ctx.enter_context(tc.tile_pool(name="consts", bufs=1))
        small = ctx.enter_context(tc.tile_pool(name="small", bufs=2))

        xT = consts.tile([P, NB, S], BF, tag="xT")
        xn = consts.tile([P, SP, D], F8, tag="xn")
        wbig = consts.tile([P, NB, D], BF, tag="wbig")
        wup16 = consts.tile([P, NB, D], BF, tag="wup16")
        crowf = consts.tile([1, D], BF, tag="crowf")
        lng_b = consts.tile([P, D], BF, tag="lng")
        lnb_b = consts.tile([P, D], BF, tag="lnb")
        qav = consts.tile([P, NB], FP, tag="qav")
        kav = consts.tile([P, NB], FP, tag="kav")
        pav = consts.tile([P, NB], FP, tag="pav")
        bv16 = consts.tile([P, NB], BF, tag="bv16")
        id_bf = consts.tile([P, P], BF, tag="id_bf")
        make_identity(nc, id_bf[:])
        ones16 = consts.tile([1, H], BF, tag="ones16")
        nc.vector.memset(ones16[:], 1.0)
        ones128 = consts.tile([1, P], BF, tag="ones128")
        nc.vector.memset(ones128[:], 1.0)
        eps_t = consts.tile([P, 1], FP, tag="eps")
        nc.vector.memset(eps_t[:], EPS)

        # =========================================================
        # Phases A-C
        # =========================================================
        with tc.tile_pool(name="wpool", bufs=3) as wp:
          with tc.tile_pool(name="phAB", bufs=1) as phab, \
               tc.tile_pool(name="spa1", bufs=1) as spa1, \
               tc.tile_pool(name="sp2", bufs=2) as sp2:

            # ---- small parameter loads (gpsimd queue) ----
            awT = phab.tile([P, SP, H], BF, tag="awT")
            bwT = phab.tile([P, SP, H], BF, tag="bwT")
            asums = phab.tile([H, 2 * NSC], FP, tag="asums")
            bsums = phab.tile([H, 2 * NSC], FP, tag="bsums")
            mask16 = phab.tile([1, S], BF, tag="mask16")
            nc.sync.dma_start(out=mask16[:], in_=mask_d[:])
            wqa = phab.tile([P, NB, H], BF, tag="wqa")
            nc.gpsimd.dma_start(out=wqa[:], in_=wqa_d[:])
            wbs = phab.tile([P, NB, H], BF, tag="wbs")
            nc.gpsimd.dma_start(out=wbs[:], in_=wbs_d[:])
            ca = phab.tile([H, 1], FP, tag="ca")
            nc.gpsimd.dma_start(out=ca[:], in_=ca_d[:])
            bb = phab.tile([H, 1], FP, tag="bb")
            nc.gpsimd.dma_start(out=bb[:], in_=bb_d[:])
            bqP = phab.tile([P, NB], FP, tag="bqP")
            nc.gpsimd.dma_start(out=bqP[:], in_=bqP_d[:])
            bkP = phab.tile([P, NB], FP, tag="bkP")
            nc.gpsimd.dma_start(out=bkP[:], in_=bkP_d[:])
            bk16 = phab.tile([P, NB], BF, tag="bk16")
            nc.gpsimd.dma_start(out=bk16[:], in_=bk16_d[:])
            nc.gpsimd.dma_start(out=bv16[:], in_=bv16_d[:])
            buwobo = phab.tile([1, D], FP, tag="buwobo")
            nc.gpsimd.dma_start(out=buwobo[:], in_=buwobo_d[:])
            for src, dst in ((lng_d, lng_b), (lnb_d, lnb_b)):
                t = phab.tile([1, D], FP, tag="lrow")
                nc.gpsimd.dma_start(out=t[:], in_=src[:])
                t16 = phab.tile([1, D], BF, tag="lrow16")
                nc.vector.tensor_copy(t16[:], t[:])
                nc.gpsimd.partition_broadcast(dst[:], t16[:1, :])

            # big-weight rotation (3 slots): wq(0) wkT(1) wk(2) wvT(0) wuwo(1)
            wq8 = wp.tile([P, NB, D], F8, tag="w")
            nc.gpsimd.dma_start(out=wq8[:], in_=wq_d[:])
            wkT8 = wp.tile([P, NB, D], F8, tag="w")
            nc.gpsimd.dma_start(out=wkT8[:], in_=wkT_d[:])

            # ---- helpers ----
            def score_sub(wsc, biast, sums, c2, ps_sc, awpool):
                # two independent 512-wide halves in rotating psum banks so
                # the next half's matmuls never WAR-wait on the prior exp
                awcs = []
                for hf in range(2):
                    o = c2 * SC + hf * (SC // 2)
                    ps = ps_sc.tile([H, SC // 2], FP, tag="sc")
                    for k in range(NB):
                        nc.tensor.matmul(
                            ps[:], wsc[:, k, :], xT[:, k, o:o + SC // 2],
                            start=(k == 0), stop=False)
                    nc.tensor.matmul(
                        ps[:], ones16[:1, :], mask16[:1, o:o + SC // 2],
                        start=False, stop=True)
                    awc = awpool.tile([H, SC // 2], BF, tag="awc")
                    nc.scalar.activation(
                        awc[:], ps[:], AF.Exp,
                        bias=biast[:, :1], scale=1.0,
                        accum_out=sums[:, 2 * c2 + hf:2 * c2 + hf + 1])
                    awcs.append(awc)
                return awcs

            def trans_sub(awcs, awT_t, c2, ps_tp):
                for hf in range(2):
                    for i in range(SPC // 2):
                        t = c2 * SPC + hf * (SPC // 2) + i
                        tp = ps_tp.tile([P, H], BF, tag="tp")
                        nc.tensor.matmul(
                            tp[:], awcs[hf][:, i * P:(i + 1) * P],
                            id_bf[:H, :H], is_transpose=True)
                        nc.vector.tensor_copy(awT_t[:, t, :], tp[:])

            def pool_sub(awT_t, pool_ps, c2):
                for i in range(SPC):
                    t = c2 * SPC + i
                    for hf in range(2):
                        o = hf * (D // 2)
                        nc.tensor.matmul(
                            pool_ps[:, o:o + D // 2], awT_t[:, t, :],
                            xn[:, t, o:o + D // 2],
                            start=(t == 0), stop=(t == SP - 1),
                            skip_group_check=True)

            def rinv_of(sums):
                tot = small.tile([H, 1], FP, tag="tot")
                nc.vector.reduce_sum(tot[:], sums[:], axis=mybir.AxisListType.X)
                rinv = small.tile([H, 1], FP, tag="rinv")
                nc.vector.reciprocal(rinv[:], tot[:])
                return rinv

            def pooled_proj_extract(pool_ps, rinv, wnat, badd, av_t,
                                    ps_tp, ps_sc):
                """av = blockdiag((pool/sum) @ Wnat) + badd  -> [P,NB] f32."""
                aXs = spa1.tile([H, D], BF, tag="xrow")
                nc.vector.tensor_scalar_mul(aXs[:], pool_ps[:], rinv[:, :1])
                aXT = spa1.tile([P, NB, H], BF, tag="aXT")
                for j in range(NB):
                    tp = ps_tp.tile([P, H], BF, tag="tp")
                    nc.tensor.matmul(
                        tp[:], aXs[:, j * P:(j + 1) * P], id_bf[:H, :H],
                        is_transpose=True)
                    nc.vector.tensor_copy(aXT[:, j, :], tp[:])
                q2h0 = ps_sc.tile([H, SC // 2], FP, tag="sc")
                q2h1 = ps_sc.tile([H, SC // 2], FP, tag="sc")
                q2h = (q2h0, q2h1)
                for k in range(NB):
                    for hf in range(2):
                        o = hf * (D // 2)
                        nc.tensor.matmul(
                            q2h[hf][:], aXT[:, k, :],
                            wnat[:, k, o:o + D // 2],
                            start=(k == 0), stop=(k == NB - 1))
                q2s = spa1.tile([H, D], BF, tag="xrow")
                for hf in range(2):
                    nc.vector.tensor_scalar_mul(
                        q2s[:, hf * (D // 2):(hf + 1) * (D // 2)], q2h[hf][:],
                        1.0 / 16.0)
                for j in range(NB):
                    tp = ps_tp.tile([P, H], BF, tag="tp")
                    nc.tensor.matmul(
                        tp[:], q2s[:, j * P:(j + 1) * P], id_bf[:H, :H],
                        is_transpose=True)
                    nc.vector.tensor_copy(
                        av_t[0:DH, j:j + 1], tp[0:DH, 2 * j:2 * j + 1])
                    nc.vector.tensor_copy(
                        av_t[DH:P, j:j + 1], tp[DH:P, 2 * j + 1:2 * j + 2])
                nc.vector.tensor_add(av_t[:], av_t[:], badd[:])

            # =====================================================
            # Phases A & B under the score/pool psum pools
            # =====================================================
            with tc.tile_pool(name="ps_sc", bufs=2, space="PSUM") as ps_sc, \
                 tc.tile_pool(name="ps_pl", bufs=1, space="PSUM") as ps_pl, \
                 tc.tile_pool(name="ps_tp", bufs=2, space="PSUM") as ps_tp, \
                 tc.tile_pool(name="ps_wkb", bufs=1, space="PSUM") as ps_wkb, \
                 tc.tile_pool(name="ps_warm", bufs=1, space="PSUM") as ps_wm, \
                 tc.tile_pool(name="sp4", bufs=4) as sp4:

                pool_ps = ps_pl.tile([H, D], FP, tag="pool")
                warm_ps = ps_wm.tile([P, P], FP, tag="warm")

                def keep_warm(n):
                    # dependency-free matmuls that fill in-order queue gaps
                    # so the PE HAM clock gate stays at 8/8 (2.4 GHz)
                    for _ in range(n):
                        nc.tensor.matmul(warm_ps[:], id_bf[:], id_bf[:],
                                         start=True, stop=True,
                                         skip_group_check=True)

                # pre-warm the PE during the initial x DMA window
                keep_warm(48)

                # ---- Phase A: x DMA + ascore + q_av pooling ----
                # software-pipelined: score(c2) issues before trans/pool of
                # c2-1, so the in-order tensor queue never waits on the
                # scalar exp of the chunk it just scored.
                pend = None
                for c2 in range(NSC):
                    lo = c2 * SC
                    if c2 == 0:
                        hs = SC // 2
                        nc.sync.dma_start(out=xT[:, :, 0:hs],
                                          in_=xT_d.ap()[:, :, 0:hs])
                        nc.sync.dma_start(out=xT[:, :, hs:SC],
                                          in_=xT_d.ap()[:, :, hs:SC])
                    else:
                        nc.sync.dma_start(out=xT[:, :, lo:lo + SC],
                                          in_=xT_d.ap()[:, :, lo:lo + SC])
                    nc.scalar.dma_start(
                        out=xn[:, c2 * SPC:(c2 + 1) * SPC, :],
                        in_=xn_d.ap()[:, c2 * SPC:(c2 + 1) * SPC, :])
                    awc = score_sub(wqa, ca, asums, c2, ps_sc, sp4)
                    if pend is not None:
                        trans_sub(pend[0], awT, pend[1], ps_tp)
                        pool_sub(awT, pool_ps, pend[1])
                    pend = (awc, c2)
                trans_sub(pend[0], awT, pend[1], ps_tp)
                pool_sub(awT, pool_ps, pend[1])

                rinv_a = rinv_of(asums)
                keep_warm(24)
                pooled_proj_extract(pool_ps, rinv_a, wq8, bqP, qav,
                                    ps_tp, ps_sc)
                keep_warm(12)

                # ---- Phase B: bscore (Wkb from q_av) + p_av pooling ----
                wk8 = wp.tile([P, NB, D], F8, tag="w")
                nc.gpsimd.dma_start(out=wk8[:], in_=wk_d[:])

                wbp = spa1.tile([P, NB, H], BF, tag="wbp")
                for j in range(NB):
                    nc.vector.tensor_scalar_mul(
                        wbp[:, j, :], wbs[:, j, :], qav[:, j:j + 1])
                wkbp = ps_wkb.tile([P, NB, H], FP, tag="wkbp")
                for m in range(NB):
                    for k in range(NB):
                        nc.tensor.matmul(
                            wkbp[:, m, :], wkT8[:, k, m * P:(m + 1) * P],
                            wbp[:, k, :],
                            start=(k == 0), stop=(k == NB - 1),
                            skip_group_check=True)
                wkb = spa1.tile([P, NB, H], BF, tag="wkb")
                nc.scalar.activation(wkb[:], wkbp[:], AF.Identity,
                                     scale=1.0 / 16.0)
                cbp = ps_sc.tile([H, SC // 2], FP, tag="sc")
                for k in range(NB):
                    nc.tensor.matmul(
                        cbp[:, :1], wbp[:, k, :], bk16[:, k:k + 1],
                        start=(k == 0), stop=(k == NB - 1))
                cb = small.tile([H, 1], FP, tag="cbt")
                nc.vector.tensor_add(cb[:], cbp[:, :1], bb[:])

                # prefetch phase-C weights into freed rotation slots
                wvT8 = wp.tile([P, NB, D], F8, tag="w")
                nc.gpsimd.dma_start(out=wvT8[:], in_=wvT_d[:])
                wuwo8 = wp.tile([P, NB, D], F8, tag="w")
                nc.gpsimd.dma_start(out=wuwo8[:], in_=wuwo_d[:])

                pend = None
                for c2 in range(NSC):
                    awc = score_sub(wkb, cb, bsums, c2, ps_sc, sp4)
                    if pend is not None:
                        trans_sub(pend[0], bwT, pend[1], ps_tp)
                        pool_sub(bwT, pool_ps, pend[1])
                    pend = (awc, c2)
                trans_sub(pend[0], bwT, pend[1], ps_tp)
                pool_sub(bwT, pool_ps, pend[1])

                rinv_b = rinv_of(bsums)
                keep_warm(24)
                pooled_proj_extract(pool_ps, rinv_b, wk8, bkP, kav,
                                    ps_tp, ps_sc)
                keep_warm(12)
                nc.vector.tensor_mul(pav[:], qav[:], kav[:])

                # WuWo' = (pav/64)-row-scaled fp8 WuWo -> bf16
                pav64 = small.tile([P, NB], FP, tag="pav64")
                nc.vector.tensor_scalar_mul(pav64[:], pav[:], 1.0 / 64.0)
                for j in range(NB):
                    nc.vector.tensor_scalar_mul(
                        wup16[:, j, :], wuwo8[:, j, :], pav64[:, j:j + 1])
                keep_warm(10)
                if not no_crow:
                    crh0 = ps_sc.tile([H, SC // 2], FP, tag="sc")
                    crh1 = ps_sc.tile([H, SC // 2], FP, tag="sc")
                    crh = (crh0, crh1)
                    for k in range(NB):
                        for hf in range(2):
                            o = hf * (D // 2)
                            nc.tensor.matmul(
                                crh[hf][:1, :], bv16[:, k:k + 1],
                                wup16[:, k, o:o + D // 2],
                                start=(k == 0), stop=(k == NB - 1))
                    for hf in range(2):
                        o = hf * (D // 2)
                        nc.vector.tensor_add(crowf[:, o:o + D // 2],
                                             crh[hf][:1, :],
                                             buwobo[:, o:o + D // 2])

          # =====================================================
          # Phase C: W_big = (WvT.T . pav) @ WuWo' + (WqWo + I)
          # =====================================================
          with tc.tile_pool(name="wstream", bufs=2) as ws, \
               tc.tile_pool(name="ps_wb", bufs=2, space="PSUM") as ps_wb:
                for m in range(NB):
                    wqwo_m = ws.tile([P, D], BF, tag="wqwom")
                    nc.gpsimd.dma_start(out=wqwo_m[:],
                                        in_=wqwo_d.ap()[:, m, :])
                    ps = ps_wb.tile([P, D], FP, tag="wbps")
                    for k in range(NB):
                        for hf in range(2):
                            o = hf * (D // 2)
                            nc.tensor.matmul(
                                ps[:, o:o + D // 2],
                                wvT8[:, k, m * P:(m + 1) * P],
                                wup16[:, k, o:o + D // 2],
                                start=(k == 0), stop=(k == NB - 1),
                                skip_group_check=True)
                    nc.vector.scalar_tensor_tensor(
                        wbig[:, m, :], ps[:], 1.0 / 16.0, wqwo_m[:],
                        op0=AL.mult, op1=AL.add)

        # =========================================================
        # Phase D: out = LN(x@(W_big+I) + crow) * g + b
        # =========================================================
        with tc.tile_pool(name="spD", bufs=3) as sp, \
             tc.tile_pool(name="ps_nat", bufs=3, space="PSUM") as ps_nat:
            for sm in range(SP):
                pn = ps_nat.tile([P, D], FP, tag="nat")
                if not no_crow:
                    for hf in range(2):
                        o = hf * (D // 2)
                        nc.tensor.matmul(
                            pn[:, o:o + D // 2], ones128[:1, :],
                            crowf[:1, o:o + D // 2],
                            start=True, stop=False, skip_group_check=True)
                for k in range(NB):
                    for hf in range(2):
                        o = hf * (D // 2)
                        nc.tensor.matmul(
                            pn[:, o:o + D // 2],
                            xT[:, k, sm * P:(sm + 1) * P],
                            wbig[:, k, o:o + D // 2],
                            start=(no_crow and k == 0), stop=(k == NB - 1),
                            skip_group_check=True)
                # LN epilogue split across scalar/vector/gpsimd so no
                # engine exceeds the tensor engine's per-block time
                y16 = sp.tile([P, D], BF, tag="y16")
                sy = small.tile([P, 1], FP, tag="sy")
                nc.scalar.activation(y16[:], pn[:], AF.Identity,
                                     accum_out=sy[:])
                ysq = sp.tile([P, D], BF, tag="ysq")
                s2 = small.tile([P, 1], FP, tag="s2")
                nc.scalar.activation(ysq[:], pn[:], AF.Square,
                                     accum_out=s2[:])
                mean = small.tile([P, 1], FP, tag="mean")
                nc.vector.tensor_scalar_mul(mean[:], sy[:], 1.0 / D)
                msq = small.tile([P, 1], FP, tag="msq")
                nc.vector.tensor_mul(msq[:], mean[:], mean[:])
                var = small.tile([P, 1], FP, tag="var")
                nc.vector.scalar_tensor_tensor(
                    var[:], s2[:], 1.0 / D, msq[:],
                    op0=AL.mult, op1=AL.subtract)
                sq = small.tile([P, 1], FP, tag="sq")
                nc.scalar.activation(sq[:], var[:], AF.Sqrt,
                                     bias=eps_t[:, :1], scale=1.0)
                rstd = small.tile([P, 1], FP, tag="rstd")
                nc.vector.reciprocal(rstd[:], sq[:])
                t16 = sp.tile([P, D], BF, tag="t16")
                nc.vector.scalar_tensor_tensor(
                    t16[:], y16[:], mean[:, :1], lng_b[:],
                    op0=AL.subtract, op1=AL.mult)
                outt = sp.tile([P, D], FP, tag="outt")
                nc.vector.scalar_tensor_tensor(
                    outt[:], t16[:], rstd[:, :1], lnb_b[:],
                    op0=AL.mult, op1=AL.add)
                nc.sync.dma_start(out=out_d[sm * P:(sm + 1) * P, :],
                                  in_=outt[:])

    nc.compile()
    return nc


def _install_ntff_hook_shim():
    """The agent image's antenv lacks axon_hooks, so trace=True degrades.
    Recreate the hook from the boot helper so neuron-profile works."""
    import types
    try:
        import antenv.axon_hooks  # noqa: F401
        return
    except ImportError:
        pass
    try:
        import antenv
        from trn_agent_boot.trn_boot import _ntff_profile_via_ctypes
        hook = _ntff_profile_via_ctypes("/opt/axon/libaxon_pjrt.so")
        mod = types.ModuleType("antenv.axon_hooks")
        mod._hook = hook
        mod.get_axon_ntff_profile_hook = lambda: mod._hook
        mod.set_axon_ntff_profile_hook = lambda h: setattr(mod, "_hook", h)
        sys.modules["antenv.axon_hooks"] = mod
        antenv.axon_hooks = mod
    except Exception as e:  # tracing is best-effort
        print(f"ntff hook shim failed: {e}", file=sys.stderr)


def _get_compiled(no_crow):
    key = ("nc", no_crow)
    if key not in _COMPILED:
        _COMPILED[key] = _build(no_crow)
    return _COMPILED[key]


def kernel(x, mask, Wq, bq, Wk, bk, Wv, bv, Wa, ba, Wb, bb, Wu, bu, Wo, bo,
           ln_g, ln_b):
    global LAST_EXEC_TIME_NS
    import ml_dtypes
    from concourse.bass_utils import run_bass_kernel_spmd

    bf16 = ml_dtypes.bfloat16
    f8 = ml_dtypes.float8_e4m3
    f32 = lambda a: np.ascontiguousarray(np.asarray(a, dtype=np.float32))

    x = f32(x)
    B = x.shape[0]
    assert B == NCORES and x.shape == (B, S, D)
    mask = f32(mask).reshape(B, S)
    Wq, Wk, Wv, Wu, Wo = f32(Wq), f32(Wk), f32(Wv), f32(Wu), f32(Wo)
    Wa, Wb = f32(Wa), f32(Wb)
    bq, bk, bv, ba, bb_, bu, bo = (f32(v).ravel() for v in
                                   (bq, bk, bv, ba, bb, bu, bo))
    ln_g, ln_b = f32(ln_g).reshape(1, D), f32(ln_b).reshape(1, D)

    # ---- host-side weight folding (f32) ----
    Wqa = (Wq @ Wa) * SCALE                      # [D, H]
    ca = ((bq @ Wa) * SCALE + ba).reshape(H, 1)
    WuWo = Wu @ Wo                               # [D, D]
    WqWo = Wq @ Wo + np.eye(D, dtype=np.float32)  # residual folded in
    buwobo = (bu @ Wo + bo).reshape(1, D)

    dmaj = lambda M: np.ascontiguousarray(
        M.reshape(NB, P, -1).transpose(1, 0, 2))     # [D, X] -> [P, NB, X]
    vP = lambda v: np.ascontiguousarray(v.reshape(NB, P).T)  # [D] -> [P, NB]

    shared = {
        "wqa": dmaj(Wqa).astype(bf16),
        "wbs": dmaj(Wb * SCALE).astype(bf16),
        "ca": ca,
        "bb": bb_.reshape(H, 1),
        "wq8": (16.0 * dmaj(Wq)).astype(f8),
        "wk8": (16.0 * dmaj(Wk)).astype(f8),
        "wkT8": (16.0 * dmaj(np.ascontiguousarray(Wk.T))).astype(f8),
        "wvT8": (16.0 * dmaj(np.ascontiguousarray(Wv.T))).astype(f8),
        "wuwo8": (64.0 * dmaj(WuWo)).astype(f8),
        "wqwo16": dmaj(WqWo).astype(bf16),
        "bqP": vP(bq),
        "bkP": vP(bk),
        "bk16P": vP(bk).astype(bf16),
        "bv16P": vP(bv).astype(bf16),
        "buwobo": buwobo,
        "ln_g": ln_g,
        "ln_b": ln_b,
    }

    no_crow = not (np.any(bv) or np.any(bu) or np.any(bo))
    nc = _get_compiled(no_crow)

    in_maps = []
    for i in range(B):
        xT16 = np.ascontiguousarray(x[i].T).astype(bf16)  # [D, S]
        m = {
            "xT16": np.ascontiguousarray(
                xT16.reshape(NB, P, S).transpose(1, 0, 2)),
            "xn8": np.ascontiguousarray(
                x[i].reshape(SP, P, D).transpose(1, 0, 2)).astype(f8),
            "mask16": mask[i:i + 1].astype(bf16),
        }
        m.update(shared)
        in_maps.append(m)

    trace = bool(int(os.environ.get("KERNEL_TRACE", "0")))
    if trace:
        _install_ntff_hook_shim()
    res = run_bass_kernel_spmd(nc, in_maps, core_ids=list(range(NCORES)),
                               trace=trace)
    LAST_EXEC_TIME_NS = res.exec_time_ns
    out = np.stack([res.results[i]["out"] for i in range(B)], axis=0)
    return out.astype(np.float32)


if __name__ == "__main__":
    np.random.seed(0)
    ins = {
        "x": np.random.randn(8, S, D).astype(np.float32),
        "mask": np.zeros((8, 1, S), np.float32),
    }
    std = 0.02
    for n, shp in (("Wq", (D, D)), ("Wk", (D, D)), ("Wv", (D, D)),
                   ("Wa", (D, H)), ("Wb", (D, H)), ("Wu", (D, D)),
                   ("Wo", (D, D))):
        ins[n] = (std * np.random.randn(*shp)).astype(np.float32)
    for n, shp in (("bq", (D,)), ("bk", (D,)), ("bv", (D,)), ("ba", (H,)),
                   ("bb", (H,)), ("bu", (D,)), ("bo", (D,)), ("ln_b", (D,))):
        ins[n] = np.zeros(shp, np.float32)
    ins["ln_g"] = np.ones((D,), np.float32)
    out = kernel(**ins)
    print("out", out.shape, out.dtype, float(np.abs(out).mean()))


# revision 55
# speedup vs baseline: 1.1749x; 1.0608x over previous
"""Trainium2 Bass kernel for the AFT-style attention module (v4).

Reference math (per batch element, S=4096, D=1024, H=16, dh=64):
    q = x@Wq+bq ; k = x@Wk+bk ; v = x@Wv+bv
    aw    = softmax(((q@Wa+ba)*s).T + mask)          # [H,S]
    q_av  = blockdiag(aw @ q)                        # [D]
    p     = k * q_av
    bw    = softmax(((p@Wb+bb)*s).T + mask)          # [H,S]
    p_av  = blockdiag(bw @ p)                        # [D]
    attn  = ((p_av * v)@Wu+bu + q) @ Wo + bo
    out   = LayerNorm(x + attn) * ln_g + ln_b

Algebraic refactor (exact; 2.3e-7 vs reference in f64):
    ascore = x @ Wqa + ca        Wqa=(Wq@Wa)*s, ca=(bq@Wa)*s+ba      (host)
    q_av   = blockdiag((aw@x) @ Wq + bq)
    bscore = x @ Wkb + cb        Wkb=(Wk . q_av) @ (Wb*s)          (device)
    p_av   = q_av * blockdiag((bw@x) @ Wk + bk)
    y      = x @ W_big + crow                  (residual folded: +I)
      W_big = (WvT.T . p_av) @ (Wu@Wo) + (Wq@Wo + I)               (host)
      crow  = (bv*p_av)@(Wu@Wo) + bu@Wo + bo
    out    = LN(y)*ln_g + ln_b

Five [S,D]@[D,D] streaming GEMMs collapse to ONE (+ one runtime [D,D]@[D,D]
and tiny score/pool matmuls).  bf16 operands / fp32 PSUM; pooling rhs is
fp8 (q_av/p_av influence the output at the 1e-4 level).  Simulated rel-err
2.9e-3 (gate 2e-2).  All wide matmuls use N=1024 bf16 moving operands
(one instruction per 2-bank psum row) to amortize the ~219-cycle
per-matmul overhead.  LN statistics run on the scalar engine via
activation accumulators; softmax is unnormalized exp with 1/sum folded
into the pooled rows.

Sharding: pure data-parallel, batch B=8 -> 8 NeuronCores, no collectives.
"""

import os

os.environ.setdefault("MYCRO_LOCAL_CACHE", "1")

import sys

if "/opt/trn_rl_repo" not in sys.path:
    sys.path.insert(0, "/opt/trn_rl_repo")

import numpy as np

S = 4096
D = 1024
H = 16
DH = 64
P = 128
NB = D // P          # 8 d-blocks of 128
SP = S // P          # 32 s-blocks of 128
SC = 1024            # score/pool sub-chunk (= DMA chunk)
NSC = S // SC        # 4
SPC = SC // P        # 8 s-blocks per sub-chunk
SCALE = float((D / H) ** -0.5)   # 0.125
EPS = 1e-6
NCORES = 8

LAST_EXEC_TIME_NS = None
_COMPILED = {}


def _build(no_crow):
    import concourse.bass as bass
    import concourse.mybir as mybir
    import concourse.tile as tile
    from concourse import bacc
    from concourse.masks import make_identity
    from contextlib import ExitStack

    FP = mybir.dt.float32
    BF = mybir.dt.bfloat16
    F8 = mybir.dt.float8e4
    AL = mybir.AluOpType
    AF = mybir.ActivationFunctionType

    nc = bacc.Bacc("TRN2", target_bir_lowering=False, debug=False)

    # ---------------- external I/O (per-core shard shapes) ----------------
    xT_d = nc.declare_dram_parameter("xT16", [P, NB, S], BF, isOutput=False)
    xn_d = nc.declare_dram_parameter("xn8", [P, SP, D], F8, isOutput=False)
    mask_d = nc.declare_dram_parameter("mask16", [1, S], BF, isOutput=False)
    wqa_d = nc.declare_dram_parameter("wqa", [P, NB, H], BF, isOutput=False)
    wbs_d = nc.declare_dram_parameter("wbs", [P, NB, H], BF, isOutput=False)
    ca_d = nc.declare_dram_parameter("ca", [H, 1], FP, isOutput=False)
    bb_d = nc.declare_dram_parameter("bb", [H, 1], FP, isOutput=False)
    # fp8 weights on the q_av/p_av/correction paths (host-scaled; the
    # inverse scales fold into drain ops)
    wq_d = nc.declare_dram_parameter("wq8", [P, NB, D], F8, isOutput=False)
    wk_d = nc.declare_dram_parameter("wk8", [P, NB, D], F8, isOutput=False)
    wkT_d = nc.declare_dram_parameter("wkT8", [P, NB, D], F8, isOutput=False)
    wvT_d = nc.declare_dram_parameter("wvT8", [P, NB, D], F8, isOutput=False)
    wuwo_d = nc.declare_dram_parameter("wuwo8", [P, NB, D], F8, isOutput=False)
    wqwo_d = nc.declare_dram_parameter("wqwo16", [P, NB, D], BF, isOutput=False)
    bqP_d = nc.declare_dram_parameter("bqP", [P, NB], FP, isOutput=False)
    bkP_d = nc.declare_dram_parameter("bkP", [P, NB], FP, isOutput=False)
    bk16_d = nc.declare_dram_parameter("bk16P", [P, NB], BF, isOutput=False)
    bv16_d = nc.declare_dram_parameter("bv16P", [P, NB], BF, isOutput=False)
    buwobo_d = nc.declare_dram_parameter("buwobo", [1, D], FP, isOutput=False)
    lng_d = nc.declare_dram_parameter("ln_g", [1, D], FP, isOutput=False)
    lnb_d = nc.declare_dram_parameter("ln_b", [1, D], FP, isOutput=False)
    out_d = nc.declare_dram_parameter("out", [S, D], FP, isOutput=True)

    with tile.TileContext(nc) as tc, ExitStack() as ctx:
        # ------------- whole-kernel pools -------------
        consts = ctx.enter_context(tc.tile_pool(name="consts", bufs=1))
        small = ctx.enter_context(tc.tile_pool(name="small", bufs=2))

        xT = consts.tile([P, NB, S], BF, tag="xT")
        xn = consts.tile([P, SP, D], F8, tag="xn")
        wbig = consts.tile([P, NB, D], BF, tag="wbig")
        wup16 = consts.tile([P, NB, D], BF, tag="wup16")
        crowf = consts.tile([1, D], BF, tag="crowf")
        lng_b = consts.tile([P, D], BF, tag="lng")
        lnb_b = consts.tile([P, D], BF, tag="lnb")
        qav = consts.tile([P, NB], FP, tag="qav")
        kav = consts.tile([P, NB], FP, tag="kav")
        pav = consts.tile([P, NB], FP, tag="pav")
        bv16 = consts.tile([P, NB], BF, tag="bv16")
        id_bf = consts.tile([P, P], BF, tag="id_bf")
        make_identity(nc, id_bf[:])
        ones16 = consts.tile([1, H], BF, tag="ones16")
        nc.vector.memset(ones16[:], 1.0)
        ones128 = consts.tile([1, P], BF, tag="ones128")
        nc.vector.memset(ones128[:], 1.0)
        eps_t = consts.tile([P, 1], FP, tag="eps")
        nc.vector.memset(eps_t[:], EPS)

        # =========================================================
        # Phases A-C
        # =========================================================
        with tc.tile_pool(name="wpool", bufs=3) as wp:
          with tc.tile_pool(name="phAB", bufs=1) as phab, \
               tc.tile_pool(name="spa1", bufs=1) as spa1, \
               tc.tile_pool(name="sp2", bufs=2) as sp2:

            # ---- small parameter loads (gpsimd queue) ----
            awT = phab.tile([P, SP, H], BF, tag="awT")
            bwT = phab.tile([P, SP, H], BF, tag="bwT")
            asums = phab.tile([H, 2 * NSC], FP, tag="asums")
            bsums = phab.tile([H, 2 * NSC], FP, tag="bsums")
            mask16 = phab.tile([1, S], BF, tag="mask16")
            nc.sync.dma_start(out=mask16[:], in_=mask_d[:])
            wqa = phab.tile([P, NB, H], BF, tag="wqa")
            nc.gpsimd.dma_start(out=wqa[:], in_=wqa_d[:])
            wbs = phab.tile([P, NB, H], BF, tag="wbs")
            nc.gpsimd.dma_start(out=wbs[:], in_=wbs_d[:])
            ca = phab.tile([H, 1], FP, tag="ca")
            nc.gpsimd.dma_start(out=ca[:], in_=ca_d[:])
            bb = phab.tile([H, 1], FP, tag="bb")
            nc.gpsimd.dma_start(out=bb[:], in_=bb_d[:])
            bqP = phab.tile([P, NB], FP, tag="bqP")
            nc.gpsimd.dma_start(out=bqP[:], in_=bqP_d[:])
            bkP = phab.tile([P, NB], FP, tag="bkP")
            nc.gpsimd.dma_start(out=bkP[:], in_=bkP_d[:])
            bk16 = phab.tile([P, NB], BF, tag="bk16")
            nc.gpsimd.dma_start(out=bk16[:], in_=bk16_d[:])
            nc.gpsimd.dma_start(out=bv16[:], in_=bv16_d[:])
            buwobo = phab.tile([1, D], FP, tag="buwobo")
            nc.gpsimd.dma_start(out=buwobo[:], in_=buwobo_d[:])
            for src, dst in ((lng_d, lng_b), (lnb_d, lnb_b)):
                t = phab.tile([1, D], FP, tag="lrow")
                nc.gpsimd.dma_start(out=t[:], in_=src[:])
                t16 = phab.tile([1, D], BF, tag="lrow16")
                nc.vector.tensor_copy(t16[:], t[:])
                nc.gpsimd.partition_broadcast(dst[:], t16[:1, :])

            # big-weight rotation (3 slots): wq(0) wkT(1) wk(2) wvT(0) wuwo(1)
            wq8 = wp.tile([P, NB, D], F8, tag="w")
            nc.gpsimd.dma_start(out=wq8[:], in_=wq_d[:])
            wkT8 = wp.tile([P, NB, D], F8, tag="w")
            nc.gpsimd.dma_start(out=wkT8[:], in_=wkT_d[:])

            # ---- helpers ----
            def score_sub(wsc, biast, sums, c2, ps_sc, awpool):
                # two independent 512-wide halves in rotating psum banks so
                # the next half's matmuls never WAR-wait on the prior exp
                awcs = []
                for hf in range(2):
                    o = c2 * SC + hf * (SC // 2)
                    ps = ps_sc.tile([H, SC // 2], FP, tag="sc")
                    for k in range(NB):
                        nc.tensor.matmul(
                            ps[:], wsc[:, k, :], xT[:, k, o:o + SC // 2],
                            start=(k == 0), stop=False)
                    nc.tensor.matmul(
                        ps[:], ones16[:1, :], mask16[:1, o:o + SC // 2],
                        start=False, stop=True)
                    awc = awpool.tile([H, SC // 2], BF, tag="awc")
                    nc.scalar.activation(
                        awc[:], ps[:], AF.Exp,
                        bias=biast[:, :1], scale=1.0,
                        accum_out=sums[:, 2 * c2 + hf:2 * c2 + hf + 1])
                    awcs.append(awc)
                return awcs

            def trans_sub(awcs, awT_t, c2, ps_tp):
                for hf in range(2):
                    for i in range(SPC // 2):
                        t = c2 * SPC + hf * (SPC // 2) + i
                        tp = ps_tp.tile([P, H], BF, tag="tp")
                        nc.tensor.matmul(
                            tp[:], awcs[hf][:, i * P:(i + 1) * P],
                            id_bf[:H, :H], is_transpose=True)
                        nc.vector.tensor_copy(awT_t[:, t, :], tp[:])

            def pool_sub(awT_t, pool_ps, c2):
                for i in range(SPC):
                    t = c2 * SPC + i
                    for hf in range(2):
                        o = hf * (D // 2)
                        nc.tensor.matmul(
                            pool_ps[:, o:o + D // 2], awT_t[:, t, :],
                            xn[:, t, o:o + D // 2],
                            start=(t == 0), stop=(t == SP - 1),
                            skip_group_check=True)

            def rinv_of(sums):
                tot = small.tile([H, 1], FP, tag="tot")
                nc.vector.reduce_sum(tot[:], sums[:], axis=mybir.AxisListType.X)
                rinv = small.tile([H, 1], FP, tag="rinv")
                nc.vector.reciprocal(rinv[:], tot[:])
                return rinv

            def pooled_proj_extract(pool_ps, rinv, wnat, badd, av_t,
                                    ps_tp, ps_sc):
                """av = blockdiag((pool/sum) @ Wnat) + badd  -> [P,NB] f32."""
                aXs = spa1.tile([H, D], BF, tag="xrow")
                nc.vector.tensor_scalar_mul(aXs[:], pool_ps[:], rinv[:, :1])
                aXT = spa1.tile([P, NB, H], BF, tag="aXT")
                for j in range(NB):
                    tp = ps_tp.tile([P, H], BF, tag="tp")
                    nc.tensor.matmul(
                        tp[:], aXs[:, j * P:(j + 1) * P], id_bf[:H, :H],
                        is_transpose=True)
                    nc.vector.tensor_copy(aXT[:, j, :], tp[:])
                q2h0 = ps_sc.tile([H, SC // 2], FP, tag="sc")
                q2h1 = ps_sc.tile([H, SC // 2], FP, tag="sc")
                q2h = (q2h0, q2h1)
                for k in range(NB):
                    for hf in range(2):
                        o = hf * (D // 2)
                        nc.tensor.matmul(
                            q2h[hf][:], aXT[:, k, :],
                            wnat[:, k, o:o + D // 2],
                            start=(k == 0), stop=(k == NB - 1))
                q2s = spa1.tile([H, D], BF, tag="xrow")
                for hf in range(2):
                    nc.vector.tensor_scalar_mul(
                        q2s[:, hf * (D // 2):(hf + 1) * (D // 2)], q2h[hf][:],
                        1.0 / 16.0)
                for j in range(NB):
                    tp = ps_tp.tile([P, H], BF, tag="tp")
                    nc.tensor.matmul(
                        tp[:], q2s[:, j * P:(j + 1) * P], id_bf[:H, :H],
                        is_transpose=True)
                    nc.vector.tensor_copy(
                        av_t[0:DH, j:j + 1], tp[0:DH, 2 * j:2 * j + 1])
                    nc.vector.tensor_copy(
                        av_t[DH:P, j:j + 1], tp[DH:P, 2 * j + 1:2 * j + 2])
                nc.vector.tensor_add(av_t[:], av_t[:], badd[:])

            # =====================================================
            # Phases A & B under the score/pool psum pools
            # =====================================================
            with tc.tile_pool(name="ps_sc", bufs=2, space="PSUM") as ps_sc, \
                 tc.tile_pool(name="ps_pl", bufs=1, space="PSUM") as ps_pl, \
                 tc.tile_pool(name="ps_tp", bufs=2, space="PSUM") as ps_tp, \
                 tc.tile_pool(name="ps_wkb", bufs=1, space="PSUM") as ps_wkb, \
                 tc.tile_pool(name="ps_warm", bufs=1, space="PSUM") as ps_wm, \
                 tc.tile_pool(name="sp4", bufs=4) as sp4:

                pool_ps = ps_pl.tile([H, D], FP, tag="pool")
                warm_ps = ps_wm.tile([P, P], FP, tag="warm")

                def keep_warm(n):
                    # dependency-free matmuls that fill in-order queue gaps
                    # so the PE HAM clock gate stays at 8/8 (2.4 GHz)
                    for _ in range(n):
                        nc.tensor.matmul(warm_ps[:], id_bf[:], id_bf[:],
                                         start=True, stop=True,
                                         skip_group_check=True)

                # pre-warm the PE during the initial x DMA window
                keep_warm(20)

                # ---- Phase A: x DMA + ascore + q_av pooling ----
                # software-pipelined: score(c2) issues before trans/pool of
                # c2-1, so the in-order tensor queue never waits on the
                # scalar exp of the chunk it just scored.
                pend = None
                for c2 in range(NSC):
                    lo = c2 * SC
                    if c2 == 0:
                        hs = SC // 2
                        nc.sync.dma_start(out=xT[:, :, 0:hs],
                                          in_=xT_d.ap()[:, :, 0:hs])
                        nc.sync.dma_start(out=xT[:, :, hs:SC],
                                          in_=xT_d.ap()[:, :, hs:SC])
                    else:
                        nc.sync.dma_start(out=xT[:, :, lo:lo + SC],
                                          in_=xT_d.ap()[:, :, lo:lo + SC])
                    nc.scalar.dma_start(
                        out=xn[:, c2 * SPC:(c2 + 1) * SPC, :],
                        in_=xn_d.ap()[:, c2 * SPC:(c2 + 1) * SPC, :])
                    awc = score_sub(wqa, ca, asums, c2, ps_sc, sp4)
                    if pend is not None:
                        trans_sub(pend[0], awT, pend[1], ps_tp)
                        pool_sub(awT, pool_ps, pend[1])
                    pend = (awc, c2)
                trans_sub(pend[0], awT, pend[1], ps_tp)
                pool_sub(awT, pool_ps, pend[1])

                rinv_a = rinv_of(asums)
                keep_warm(24)
                pooled_proj_extract(pool_ps, rinv_a, wq8, bqP, qav,
                                    ps_tp, ps_sc)
                keep_warm(12)

                # ---- Phase B: bscore (Wkb from q_av) + p_av pooling ----
                wk8 = wp.tile([P, NB, D], F8, tag="w")
                nc.gpsimd.dma_start(out=wk8[:], in_=wk_d[:])

                wbp = spa1.tile([P, NB, H], BF, tag="wbp")
                for j in range(NB):
                    nc.vector.tensor_scalar_mul(
                        wbp[:, j, :], wbs[:, j, :], qav[:, j:j + 1])
                wkbp = ps_wkb.tile([P, NB, H], FP, tag="wkbp")
                for m in range(NB):
                    for k in range(NB):
                        nc.tensor.matmul(
                            wkbp[:, m, :], wkT8[:, k, m * P:(m + 1) * P],
                            wbp[:, k, :],
                            start=(k == 0), stop=(k == NB - 1),
                            skip_group_check=True)
                wkb = spa1.tile([P, NB, H], BF, tag="wkb")
                nc.scalar.activation(wkb[:], wkbp[:], AF.Identity,
                                     scale=1.0 / 16.0)
                cbp = ps_sc.tile([H, SC // 2], FP, tag="sc")
                for k in range(NB):
                    nc.tensor.matmul(
                        cbp[:, :1], wbp[:, k, :], bk16[:, k:k + 1],
                        start=(k == 0), stop=(k == NB - 1))
                cb = small.tile([H, 1], FP, tag="cbt")
                nc.vector.tensor_add(cb[:], cbp[:, :1], bb[:])

                # prefetch phase-C weights into freed rotation slots
                wvT8 = wp.tile([P, NB, D], F8, tag="w")
                nc.gpsimd.dma_start(out=wvT8[:], in_=wvT_d[:])
                wuwo8 = wp.tile([P, NB, D], F8, tag="w")
                nc.gpsimd.dma_start(out=wuwo8[:], in_=wuwo_d[:])

                pend = None
                for c2 in range(NSC):
                    awc = score_sub(wkb, cb, bsums, c2, ps_sc, sp4)
                    if pend is not None:
                        trans_sub(pend[0], bwT, pend[1], ps_tp)
                        pool_sub(bwT, pool_ps, pend[1])
                    pend = (awc, c2)
                trans_sub(pend[0], bwT, pend[1], ps_tp)
                pool_sub(bwT, pool_ps, pend[1])

                rinv_b = rinv_of(bsums)
                keep_warm(24)
                pooled_proj_extract(pool_ps, rinv_b, wk8, bkP, kav,
                                    ps_tp, ps_sc)
                keep_warm(12)
                nc.vector.tensor_mul(pav[:], qav[:], kav[:])

                # WuWo' = (pav/64)-row-scaled fp8 WuWo -> bf16
                pav64 = small.tile([P, NB], FP, tag="pav64")
                nc.vector.tensor_scalar_mul(pav64[:], pav[:], 1.0 / 64.0)
                for j in range(NB):
                    nc.vector.tensor_scalar_mul(
                        wup16[:, j, :], wuwo8[:, j, :], pav64[:, j:j + 1])
                keep_warm(10)
                if not no_crow:
                    crh0 = ps_sc.tile([H, SC // 2], FP, tag="sc")
                    crh1 = ps_sc.tile([H, SC // 2], FP, tag="sc")
                    crh = (crh0, crh1)
                    for k in range(NB):
                        for hf in range(2):
                            o = hf * (D // 2)
                            nc.tensor.matmul(
                                crh[hf][:1, :], bv16[:, k:k + 1],
                                wup16[:, k, o:o + D // 2],
                                start=(k == 0), stop=(k == NB - 1))
                    for hf in range(2):
                        o = hf * (D // 2)
                        nc.vector.tensor_add(crowf[:, o:o + D // 2],
                                             crh[hf][:1, :],
                                             buwobo[:, o:o + D // 2])

          # =====================================================
          # Phase C: W_big = (WvT.T . pav) @ WuWo' + (WqWo + I)
          # =====================================================
          with tc.tile_pool(name="wstream", bufs=2) as ws, \
               tc.tile_pool(name="ps_wb", bufs=2, space="PSUM") as ps_wb:
                for m in range(NB):
                    wqwo_m = ws.tile([P, D], BF, tag="wqwom")
                    nc.gpsimd.dma_start(out=wqwo_m[:],
                                        in_=wqwo_d.ap()[:, m, :])
                    ps = ps_wb.tile([P, D], FP, tag="wbps")
                    for k in range(NB):
                        for hf in range(2):
                            o = hf * (D // 2)
                            nc.tensor.matmul(
                                ps[:, o:o + D // 2],
                                wvT8[:, k, m * P:(m + 1) * P],
                                wup16[:, k, o:o + D // 2],
                                start=(k == 0), stop=(k == NB - 1),
                                skip_group_check=True)
                    nc.vector.scalar_tensor_tensor(
                        wbig[:, m, :], ps[:], 1.0 / 16.0, wqwo_m[:],
                        op0=AL.mult, op1=AL.add)

        # =========================================================
        # Phase D: out = LN(x@(W_big+I) + crow) * g + b
        # =========================================================
        with tc.tile_pool(name="spD", bufs=3) as sp, \
             tc.tile_pool(name="ps_nat", bufs=3, space="PSUM") as ps_nat:
            for sm in range(SP):
                pn = ps_nat.tile([P, D], FP, tag="nat")
                if not no_crow:
                    for hf in range(2):
                        o = hf * (D // 2)
                        nc.tensor.matmul(
                            pn[:, o:o + D // 2], ones128[:1, :],
                            crowf[:1, o:o + D // 2],
                            start=True, stop=False, skip_group_check=True)
                for k in range(NB):
                    for hf in range(2):
                        o = hf * (D // 2)
                        nc.tensor.matmul(
                            pn[:, o:o + D // 2],
                            xT[:, k, sm * P:(sm + 1) * P],
                            wbig[:, k, o:o + D // 2],
                            start=(no_crow and k == 0), stop=(k == NB - 1),
                            skip_group_check=True)
                # LN epilogue split across scalar/vector/gpsimd so no
                # engine exceeds the tensor engine's per-block time
                y16 = sp.tile([P, D], BF, tag="y16")
                sy = small.tile([P, 1], FP, tag="sy")
                nc.scalar.activation(y16[:], pn[:], AF.Identity,
                                     accum_out=sy[:])
                ysq = sp.tile([P, D], BF, tag="ysq")
                s2 = small.tile([P, 1], FP, tag="s2")
                nc.scalar.activation(ysq[:], y16[:], AF.Square,
                                     accum_out=s2[:])
                mean = small.tile([P, 1], FP, tag="mean")
                nc.vector.tensor_scalar_mul(mean[:], sy[:], 1.0 / D)
                msq = small.tile([P, 1], FP, tag="msq")
                nc.vector.tensor_mul(msq[:], mean[:], mean[:])
                var = small.tile([P, 1], FP, tag="var")
                nc.vector.scalar_tensor_tensor(
                    var[:], s2[:], 1.0 / D, msq[:],
                    op0=AL.mult, op1=AL.subtract)
                sq = small.tile([P, 1], FP, tag="sq")
                nc.scalar.activation(sq[:], var[:], AF.Sqrt,
                                     bias=eps_t[:, :1], scale=1.0)
                rstd = small.tile([P, 1], FP, tag="rstd")
                nc.vector.reciprocal(rstd[:], sq[:])
                t16 = sp.tile([P, D], BF, tag="t16")
                nc.vector.scalar_tensor_tensor(
                    t16[:], y16[:], mean[:, :1], lng_b[:],
                    op0=AL.subtract, op1=AL.mult)
                outt = sp.tile([P, D], FP, tag="outt")
                nc.vector.scalar_tensor_tensor(
                    outt[:], t16[:], rstd[:, :1], lnb_b[:],
                    op0=AL.mult, op1=AL.add)
                nc.sync.dma_start(out=out_d[sm * P:(sm + 1) * P, :],
                                  in_=outt[:])

    nc.compile()
    return nc


def _install_ntff_hook_shim():
    """The agent image's antenv lacks axon_hooks, so trace=True degrades.
    Recreate the hook from the boot helper so neuron-profile works."""
    import types
    try:
        import antenv.axon_hooks  # noqa: F401
        return
    except ImportError:
        pass
    try:
        import antenv
        from trn_agent_boot.trn_boot import _ntff_profile_via_ctypes
        hook = _ntff_profile_via_ctypes("/opt/axon/libaxon_pjrt.so")
        mod = types.ModuleType("antenv.axon_hooks")
        mod._hook = hook
        mod.get_axon_ntff_profile_hook = lambda: mod._hook
        mod.set_axon_ntff_profile_hook = lambda h: setattr(mod, "_hook", h)
        sys.modules["antenv.axon_hooks"] = mod
        antenv.axon_hooks = mod
    except Exception as e:  # tracing is best-effort
        print(f"ntff hook shim failed: {e}", file=sys.stderr)


def _get_compiled(no_crow):
    key = ("nc", no_crow)
    if key not in _COMPILED:
        _COMPILED[key] = _build(no_crow)
    return _COMPILED[key]


def kernel(x, mask, Wq, bq, Wk, bk, Wv, bv, Wa, ba, Wb, bb, Wu, bu, Wo, bo,
           ln_g, ln_b):
    global LAST_EXEC_TIME_NS
    import ml_dtypes
    from concourse.bass_utils import run_bass_kernel_spmd

    bf16 = ml_dtypes.bfloat16
    f8 = ml_dtypes.float8_e4m3
    f32 = lambda a: np.ascontiguousarray(np.asarray(a, dtype=np.float32))

    x = f32(x)
    B = x.shape[0]
    assert B == NCORES and x.shape == (B, S, D)
    mask = f32(mask).reshape(B, S)
    Wq, Wk, Wv, Wu, Wo = f32(Wq), f32(Wk), f32(Wv), f32(Wu), f32(Wo)
    Wa, Wb = f32(Wa), f32(Wb)
    bq, bk, bv, ba, bb_, bu, bo = (f32(v).ravel() for v in
                                   (bq, bk, bv, ba, bb, bu, bo))
    ln_g, ln_b = f32(ln_g).reshape(1, D), f32(ln_b).reshape(1, D)

    # ---- host-side weight folding (f32) ----
    Wqa = (Wq @ Wa) * SCALE                      # [D, H]
    ca = ((bq @ Wa) * SCALE + ba).reshape(H, 1)
    WuWo = Wu @ Wo                               # [D, D]
    WqWo = Wq @ Wo + np.eye(D, dtype=np.float32)  # residual folded in
    buwobo = (bu @ Wo + bo).reshape(1, D)

    dmaj = lambda M: np.ascontiguousarray(
        M.reshape(NB, P, -1).transpose(1, 0, 2))     # [D, X] -> [P, NB, X]
    vP = lambda v: np.ascontiguousarray(v.reshape(NB, P).T)  # [D] -> [P, NB]

    shared = {
        "wqa": dmaj(Wqa).astype(bf16),
        "wbs": dmaj(Wb * SCALE).astype(bf16),
        "ca": ca,
        "bb": bb_.reshape(H, 1),
        "wq8": (16.0 * dmaj(Wq)).astype(f8),
        "wk8": (16.0 * dmaj(Wk)).astype(f8),
        "wkT8": (16.0 * dmaj(np.ascontiguousarray(Wk.T))).astype(f8),
        "wvT8": (16.0 * dmaj(np.ascontiguousarray(Wv.T))).astype(f8),
        "wuwo8": (64.0 * dmaj(WuWo)).astype(f8),
        "wqwo16": dmaj(WqWo).astype(bf16),
        "bqP": vP(bq),
        "bkP": vP(bk),
        "bk16P": vP(bk).astype(bf16),
        "bv16P": vP(bv).astype(bf16),
        "buwobo": buwobo,
        "ln_g": ln_g,
        "ln_b": ln_b,
    }

    no_crow = not (np.any(bv) or np.any(bu) or np.any(bo))
    nc = _get_compiled(no_crow)

    in_maps = []
    for i in range(B):
        xT16 = np.ascontiguousarray(x[i].T).astype(bf16)  # [D, S]
        m = {
            "xT16": np.ascontiguousarray(
                xT16.reshape(NB, P, S).transpose(1, 0, 2)),
            "xn8": np.ascontiguousarray(
                x[i].reshape(SP, P, D).transpose(1, 0, 2)).astype(f8),
            "mask16": mask[i:i + 1].astype(bf16),
        }
        m.update(shared)
        in_maps.append(m)

    trace = bool(int(os.environ.get("KERNEL_TRACE", "0")))
    if trace:
        _install_ntff_hook_shim()
    res = run_bass_kernel_spmd(nc, in_maps, core_ids=list(range(NCORES)),
                               trace=trace)
    LAST_EXEC_TIME_NS = res.exec_time_ns
    out = np.stack([res.results[i]["out"] for i in range(B)], axis=0)
    return out.astype(np.float32)


if __name__ == "__main__":
    np.random.seed(0)
    ins = {
        "x": np.random.randn(8, S, D).astype(np.float32),
        "mask": np.zeros((8, 1, S), np.float32),
    }
    std = 0.02
    for n, shp in (("Wq", (D, D)), ("Wk", (D, D)), ("Wv", (D, D)),
                   ("Wa", (D, H)), ("Wb", (D, H)), ("Wu", (D, D)),
                   ("Wo", (D, D))):
        ins[n] = (std * np.random.randn(*shp)).astype(np.float32)
    for n, shp in (("bq", (D,)), ("bk", (D,)), ("bv", (D,)), ("ba", (H,)),
                   ("bb", (H,)), ("bu", (D,)), ("bo", (D,)), ("ln_b", (D,))):
        ins[n] = np.zeros(shp, np.float32)
    ins["ln_g"] = np.ones((D,), np.float32)
    out = kernel(**ins)
    print("out", out.shape, out.dtype, float(np.abs(out).mean()))
